# revision 29
# baseline (speedup 1.0000x reference)
"""Trainium2 Bass kernel for a CrossAttentionBlock.

Reference computation (B=4, C=256, H=W=64, 4 heads, head_dim=64):
  q = Wq @ GN(x);  k = Wk @ GN(ctx);  v = Wv @ ctx        (1x1 convs)
  attn = softmax(q^T k / sqrt(hd))  per (batch, head)
  out  = x + Wo @ (v @ attn^T) + bo

Interface: 4 cores, one full batch element per core. All per-core inputs
are packed into a SINGLE f16 blob tensor (the PJRT-over-axon execute path
pays a large per-argument per-call cost, and per-byte transfer cost, so
one small blob argument beats twelve f32 tensors by ~10x end to end).
Output is a single f16 [C, HW] tensor per core.

On-chip algorithm per core (matmuls in f16, fp32 PSUM accum):
  - GroupNorm stats via ones-selector matmul (per-group sums of x and x^2),
    expanded back to per-channel affine (a, d) with a selector matmul.
  - The GN affine is FOLDED into the Q/K projections: Wq' = Wq. diag(a)
    (per-partition row scale of the resident weights) and a rank-1 bias
    qb = Wq @ d computed with an N=1 matmul, so the normalized tensors are
    never materialized.
  - S^T[e, d] = sum_c k[c,e] q[c,d] per 128-wide e-chunk so the attn @ v
    matmul needs no transpose: lhsT = [v^T | ones], so PSUM row 64
    accumulates the softmax denominator for free.  exp(S - 4) on the
    scalar engine keeps the f16 range safe (|S| <~ 8 for these inputs;
    the -4 shift cancels in the softmax ratio).
  - softmax divide via DVE reciprocal + a rank-1 fp32 ones matmul that
    replicates the per-column reciprocal across partitions.
"""

import sys

if "/opt/trn_rl_repo" not in sys.path:
    sys.path.insert(0, "/opt/trn_rl_repo")

import copy
from contextlib import ExitStack

import numpy as np

import bass_rust
import concourse.bass as bass
import concourse.mybir as mybir
import concourse.tile as tile
from concourse.bass_utils import run_bass_kernel_spmd
from concourse.vector_clock import ScopedClock

F32 = mybir.dt.float32
F16 = mybir.dt.float16

N_CORES = 4
B, C, HW = 4, 256, 4096
NH, HD = 4, 64          # heads, head dim
P = 128                 # partitions
NSUB = C // P           # channel subtiles (2)
GROUPS = 32             # groupnorm groups (16 per channel-subtile)
CH_PER_G = C // GROUPS  # 8
GN_N = CH_PER_G * HW    # elements per group (32768)
EPS = 1e-5
DJ = 1024               # main-loop d-chunk (exp granularity)
NDJ = HW // DJ          # 4
NE = HW // P            # 32 e-chunks
EXP_SHIFT = -4.0        # exp(S + EXP_SHIFT); cancels in softmax ratio
ALU = mybir.AluOpType
ACTF = mybir.ActivationFunctionType
AXX = mybir.AxisListType.X

# ---- blob layout (f16 element offsets) ----
OFF_X = 0                          # x    [C, HW] as (t p d)
OFF_CTX = OFF_X + C * HW           # ctx  [C, HW] as (t p d)
OFF_W = OFF_CTX + C * HW           # wqt, wkt, wvt, wot [C, C] as (t p o)
OFF_GNP = OFF_W + 4 * C * C        # [P, NSUB, 4] (p t f)
OFF_BO = OFF_GNP + P * NSUB * 4    # [P, NSUB] (p t)
OFF_GSEL = OFF_BO + P * NSUB       # [P, 16] (p g)
OFF_SELT = OFF_GSEL + P * 16       # [16, P] (g p)
BLOB_N = OFF_SELT + 16 * P


class SplitDrainTileContext(tile.TileContext):
    """TileContext whose exit drain splits sem waits across multiple Drain
    instructions — the walrus build in this container rejects >2 sync waits
    on a single Drain ("Too many sync wait commands")."""

    def _drain_and_barrier(self, tick_clock, wait_clock):
        drain_inst = self.nc.sync.drain()
        wait_clock.add_sem_waits(
            drain_inst.ins, ScopedClock({None: tick_clock.global_clock})
        )
        si = drain_inst.ins.sync_info
        if si is not None and si.on_wait and len(si.on_wait) > 1:
            waits = list(si.on_wait)
            si.on_wait = waits[:1]
            drain_inst.ins.sync_info = si
            for w in waits[1:]:
                extra = self.nc.sync.drain()
                extra.ins.sync_info = bass_rust.SyncInfo(on_wait=[w], on_update=[])
        self.nc.all_engine_barrier()
        popped = self.nc._tile_sem_poison_stack.pop()
        assert popped is self._sem_poison
        self.nc.clear_and_free_semaphores(list(self.sems.allocated().values()))
        self.nc.all_engine_barrier()


_NOP_TMPL = []


def _nop_template():
    if not _NOP_TMPL:
        tb = bass.Bass()
        with tb.bb("t"):
            _NOP_TMPL.append(copy.copy(tb.vector.nop().ins))
    return _NOP_TMPL[0]


def _split_excess_waits(nc, limit=1):
    """This container's walrus rejects instructions carrying more than ~2
    sync-wait commands. Spill excess waits onto same-engine NoOps inserted
    just before the overloaded instruction (waiting earlier on the same
    engine is semantics-preserving; NoOps have no dependents, so no cycles
    can form)."""
    tmpl = _nop_template()
    n = 0

    def fix(blk):
        nonlocal n
        if hasattr(blk, "instructions"):
            out = []
            changed = False
            for inst in blk.instructions:
                si = inst.sync_info
                ow = list(si.on_wait) if (si is not None and si.on_wait) else []
                lim = 1 if ("DMA" in inst.opcode or inst.opcode == "Drain") \
                    else limit
                if len(ow) > lim:
                    changed = True
                    for w in ow[:-lim]:
                        sp = copy.copy(tmpl)
                        n += 1
                        sp.name = f"I-wsp-{n}"
                        sp.engine = inst.engine
                        sp.sync_info = bass_rust.SyncInfo(on_wait=[w],
                                                          on_update=[])
                        out.append(sp)
                    si.on_wait = ow[-lim:]
                    inst.sync_info = si
                out.append(inst)
            if changed:
                blk.instructions = out
        for sub in getattr(blk, "blocks", []) or []:
            fix(sub)

    for f in nc.m.functions:
        for blk in f.blocks:
            fix(blk)
    return n


def build_module(iters: int = 1) -> bass.Bass:
    """iters > 1 repeats the whole body (for slope-based device timing)."""
    nc = bass.Bass()
    blob = nc.dram_tensor("blob", [BLOB_N], F16, kind="ExternalInput")
    out = nc.dram_tensor("out", [C, HW], F16, kind="ExternalOutput")
    with SplitDrainTileContext(nc) as tc:
        for _ in range(iters):
            _emit(nc, tc, blob, out)
    _split_excess_waits(nc)
    return nc


def _emit(nc, tc, blob, out_dr):
    x_view = blob[OFF_X:OFF_X + C * HW].rearrange("(t p d) -> p t d",
                                                  t=NSUB, p=P)
    ctx_view = blob[OFF_CTX:OFF_CTX + C * HW].rearrange("(t p d) -> p t d",
                                                        t=NSUB, p=P)
    w_views = [
        blob[OFF_W + i * C * C:OFF_W + (i + 1) * C * C].rearrange(
            "(t p o) -> p t o", t=NSUB, p=P)
        for i in range(4)
    ]
    gnp_view = blob[OFF_GNP:OFF_GNP + P * NSUB * 4].rearrange(
        "(p t f) -> p t f", p=P, t=NSUB)
    bo_view = blob[OFF_BO:OFF_BO + P * NSUB].rearrange("(a o) -> a o", a=1)
    gsel_view = blob[OFF_GSEL:OFF_GSEL + P * 16].rearrange("(p g) -> p g", p=P)
    selt_view = blob[OFF_SELT:OFF_SELT + 16 * P].rearrange("(g p) -> g p", g=16)
    out_view = out_dr[:].rearrange("(t p) d -> p t d", p=P)

    with ExitStack() as ctx:
        pw = ctx.enter_context(tc.tile_pool(name="pw", bufs=1))
        pmain = ctx.enter_context(tc.tile_pool(name="pmain", bufs=1))
        ptp = ctx.enter_context(tc.tile_pool(name="ptp", bufs=4))
        psmall = ctx.enter_context(tc.tile_pool(name="psmall", bufs=2))

        # ---- small constants (gsel first: it gates the stats matmuls) ----
        gsel_sb = pw.tile([P, 16], F16, name="gsel_sb")
        nc.sync.dma_start(gsel_sb[:], gsel_view)
        gnp_sb = pw.tile([P, NSUB, 4], F16, name="gnp_sb")
        nc.gpsimd.dma_start(gnp_sb[:], gnp_view)
        selt_sb = pw.tile([16, P], F16, name="selt_sb")
        nc.gpsimd.dma_start(selt_sb[:], selt_view)
        bo_sb = pw.tile([1, C], F16, name="bo_sb")
        nc.scalar.dma_start(bo_sb[:], bo_view)
        wq_sb = pw.tile([P, NSUB, C], F16, name="wq_sb")
        wk_sb = pw.tile([P, NSUB, C], F16, name="wk_sb")
        wv_sb = pw.tile([P, NSUB, C], F16, name="wv_sb")
        wo_sb = pw.tile([P, NSUB, C], F16, name="wo_sb")
        wmup_sb = pw.tile([P, P], F16, name="wmup_sb")
        nc.vector.memset(wmup_sb[:], 0.0)
        ones_sb = pw.tile([1, HD], F16, name="ones_sb")
        nc.vector.memset(ones_sb[:], 1.0)
        ones512_sb = pw.tile([1, 512], F16, name="ones512_sb")
        nc.vector.memset(ones512_sb[:], 1.0)
        eps_sb = pw.tile([16, 1], F32, name="eps_sb")
        nc.vector.memset(eps_sb[:], EPS)
        shift_sb = pw.tile([P, 1], F32, name="shift_sb")
        nc.vector.memset(shift_sb[:], EXP_SHIFT)

        # ---- persistent activations, chunked across three DMA queues so
        # GN stats can start early on partial data ----
        x_sb = pmain.tile([P, NSUB, HW], F16, name="x_sb")
        cb_sb = pmain.tile([P, NSUB, HW], F16, name="cb_sb")
        big = [(x_sb, x_view, 0), (x_sb, x_view, 1),
               (cb_sb, ctx_view, 0), (cb_sb, ctx_view, 1),
               (x_sb, x_view, 2), (x_sb, x_view, 3),
               (cb_sb, ctx_view, 2), (cb_sb, ctx_view, 3)]
        queues = [nc.sync, nc.gpsimd, nc.scalar]
        for qi, (dst, view, cch) in enumerate(big):
            sl = slice(cch * 1024, (cch + 1) * 1024)
            queues[qi % 3].dma_start(dst[:, :, sl], view[:, :, sl])
        # weights after the activations (first needed ~10us in)
        for i, w_sb in enumerate((wq_sb, wk_sb, wv_sb, wo_sb)):
            queues[i % 3].dma_start(w_sb[:], w_views[i])
        q_sb = pmain.tile([P, NSUB, HW], F16, name="q_sb")
        k_sb = pmain.tile([P, NSUB, HW], F16, name="k_sb")
        vt_sb = pmain.tile([P, NE, NH, 66], F16, name="vt_sb")
        ao_sb = pmain.tile([P, NSUB, HW], F16, name="ao_sb")
        stats_sb = pmain.tile([16, 8], F32, name="stats_sb")
        grp_sb = pmain.tile([P, 8], F32, name="grp_sb")
        aff_sb = pmain.tile([P, 2, NSUB, 2], F32, name="aff_sb")
        dvec_sb = pmain.tile([P, 2, NSUB, 1], F16, name="dvec_sb")
        qb_sb = pmain.tile([P, 2, NSUB], F32, name="qb_sb")

        # ============ prep phase: GN stats, fold affine, Q/K/V^T ============
        with ExitStack() as prep:
            pps = prep.enter_context(
                tc.tile_pool(name="pps", bufs=2, space="PSUM"))
            pchunk = prep.enter_context(tc.tile_pool(name="pchunk", bufs=2))

            # keep the PE continuously busy from t=0 so it reaches the full
            # 2.4GHz p-state before the real prep matmuls arrive
            for w in range(20):
                junk = pps.tile([P, 512], F32, tag="qk", name="junk")
                nc.tensor.matmul(junk[:, :P], wmup_sb[:], wmup_sb[:, :P],
                                 start=True, stop=True)

            def gn_stats(src_sb, tensor_idx):
                """Per-group sums of x and x^2 via the selector matmul,
                streamed in [P, NSUB, 512] chunks out of resident f16 data.
                Column-reduces split across DVE and Act to shorten the
                serial chain."""
                nj = HW // 512
                ps = {
                    (t, k): pps.tile([16, 512], F32, tag="stat", bufs=4,
                                     name=f"ps_stat{tensor_idx}{t}{k}")
                    for t in range(NSUB) for k in range(2)
                }
                for j in range(nj):
                    sl = slice(j * 512, (j + 1) * 512)
                    sq = pchunk.tile([P, NSUB, 512], F16, tag="sq", name="sq")
                    for t in range(NSUB):
                        nc.vector.tensor_mul(sq[:, t], src_sb[:, t, sl],
                                             src_sb[:, t, sl])
                        nc.tensor.matmul(ps[(t, 0)][:], gsel_sb[:],
                                         src_sb[:, t, sl],
                                         start=(j == 0), stop=(j == nj - 1))
                        nc.tensor.matmul(ps[(t, 1)][:], gsel_sb[:], sq[:, t],
                                         start=(j == 0), stop=(j == nj - 1))
                for t in range(NSUB):
                    for k in range(2):
                        dst = stats_sb[:, 4 * tensor_idx + 2 * t + k:
                                       4 * tensor_idx + 2 * t + k + 1]
                        if k == 0:
                            nc.vector.reduce_sum(dst, ps[(t, k)][:], axis=AXX)
                        else:
                            nc.scalar.activation(ps[(t, k)][:], ps[(t, k)][:],
                                                 ACTF.Copy, accum_out=dst)

            gn_stats(x_sb, 0)
            gn_stats(cb_sb, 1)

            # ---- group mean / rstd;  i = T*2 + t ----
            packed = psmall.tile([16, 8], F32, name="packed")
            inv_n = 1.0 / GN_N
            for T in range(2):
                for t in range(NSUB):
                    i = T * 2 + t
                    mean = packed[:, 2 * i:2 * i + 1]
                    rstd = packed[:, 2 * i + 1:2 * i + 2]
                    nc.vector.tensor_scalar_mul(
                        mean, stats_sb[:, 4 * T + 2 * t:4 * T + 2 * t + 1],
                        inv_n)
                    nc.vector.tensor_scalar_mul(
                        rstd,
                        stats_sb[:, 4 * T + 2 * t + 1:4 * T + 2 * t + 2],
                        inv_n)
                    m2 = psmall.tile([16, 1], F32, tag="m2", name="m2")
                    nc.vector.tensor_mul(m2[:], mean, mean)
                    nc.vector.tensor_sub(rstd, rstd, m2[:])
                    nc.scalar.activation(rstd, rstd, ACTF.Sqrt, bias=eps_sb[:])
                    nc.vector.reciprocal(rstd, rstd)

            # ---- V^T projection (independent of GN; overlaps the affine
            # chain below; PSUM->SBUF copies on the otherwise idle Act) ----
            nc.vector.memset(vt_sb[:, :, :, 64:65], 1.0)
            nc.vector.memset(vt_sb[:, :, :, 65:66], 0.0)
            for ec in range(NE):
                psv = pps.tile([P, 512], F32, tag="qk", name="psv")
                for t in range(NSUB):
                    nc.tensor.matmul(
                        psv[:, :C], cb_sb[:, t, ec * P:(ec + 1) * P],
                        wv_sb[:, t, :],
                        start=(t == 0), stop=(t == NSUB - 1))
                src = psv[:, :C].rearrange("p (h c) -> p h c", c=64)
                if ec % 2 == 0:
                    nc.scalar.activation(vt_sb[:, ec, :, 0:64], src,
                                         ACTF.Copy)
                else:
                    nc.vector.tensor_copy(vt_sb[:, ec, :, 0:64], src)

            # prewarm the Exp activation table as the Act engine's last prep
            # op, so the first real exp doesn't pay the table-load stall
            dmy = psmall.tile([1, 1], F16, tag="dmy", name="dmy")
            nc.scalar.activation(dmy[:], shift_sb[0:1, :], ACTF.Exp)

            # expand groups -> channels with a selector matmul (f16 operands)
            packed16 = psmall.tile([16, 8], F16, tag="p16", name="packed16")
            nc.vector.tensor_copy(packed16[:], packed[:])
            psg = pps.tile([P, 8], F32, tag="exp", bufs=1, name="psg")
            nc.tensor.matmul(psg[:], selt_sb[:], packed16[:], start=True,
                             stop=True)
            nc.vector.tensor_copy(grp_sb[:], psg[:])

            # affine: a = w * rstd ; d = b - mean * a
            for T in range(2):
                for t in range(NSUB):
                    i = T * 2 + t
                    wcol = 0 if T == 0 else 2
                    a = aff_sb[:, T, t, 0:1]
                    d = aff_sb[:, T, t, 1:2]
                    nc.vector.tensor_mul(
                        a, gnp_sb[:, t, wcol:wcol + 1],
                        grp_sb[:, 2 * i + 1:2 * i + 2])
                    tmp = psmall.tile([P, 1], F32, tag="afft", name="afft")
                    nc.vector.tensor_mul(tmp[:], grp_sb[:, 2 * i:2 * i + 1], a)
                    nc.vector.tensor_sub(
                        d, gnp_sb[:, t, wcol + 1:wcol + 2], tmp[:])
                    nc.vector.tensor_copy(dvec_sb[:, T, t], d)

            # rank-1 GN bias through the UNSCALED weights: qb = W^T d
            for T, w_sb in ((0, wq_sb), (1, wk_sb)):
                pb = pps.tile([P, NSUB], F32, tag="bias", bufs=1,
                              name=f"pbias{T}")
                for i in range(NSUB):
                    for t in range(NSUB):
                        nc.tensor.matmul(
                            pb[:, i:i + 1], w_sb[:, t, i * P:(i + 1) * P],
                            dvec_sb[:, T, t],
                            start=(t == 0), stop=(t == NSUB - 1))
                nc.vector.tensor_copy(qb_sb[:, T], pb[:])

            # fold the GN scale into the weights (per-partition row scale)
            for T, w_sb in ((0, wq_sb), (1, wk_sb)):
                for t in range(NSUB):
                    nc.vector.tensor_scalar(
                        w_sb[:, t], w_sb[:, t], aff_sb[:, T, t, 0:1], None,
                        op0=ALU.mult)

            # ---- K/Q projection chunks the main loop needs up front:
            # K(i=0) fully and Q(i=0) first two chunks (heads 0/1, dj=0) ----
            def proj_chunk(T, i, jd, pool, tag, bufs=1):
                w_sb = wq_sb if T == 0 else wk_sb
                src = x_sb if T == 0 else cb_sb
                dst = q_sb if T == 0 else k_sb
                psq = pool.tile([P, 512], F32, tag=tag, bufs=bufs, name="psq")
                for t in range(NSUB):
                    nc.tensor.matmul(
                        psq[:], w_sb[:, t, i * P:(i + 1) * P],
                        src[:, t, jd * 512:(jd + 1) * 512],
                        start=(t == 0), stop=(t == NSUB - 1))
                nc.vector.tensor_scalar(
                    dst[:, i, jd * 512:(jd + 1) * 512], psq[:],
                    qb_sb[:, T, i:i + 1], None, op0=ALU.add)

            # psq tiles go in the stat banks (free once the reduces are done)
            # so the K/Q chunks never rotate behind the V-copy stream
            for jd in range(HW // 512):
                proj_chunk(1, 0, jd, pps, "stat", bufs=4)
            for jd in range(2):
                proj_chunk(0, 0, jd, pps, "stat", bufs=4)

        # ================= attention main loop =================
        # Remaining K/Q projection chunks stream into the early e-chunk
        # iterations (even ecs) where the PE has slack vs the Act engine.
        # Ordered so each chunk lands well before its consumer head.
        # Divide/Wo work is DEFERRED into the next head's e-chunk stream so
        # the Act engine (the bottleneck) never waits on the PE-side pieces
        # of the softmax divide at head boundaries.  po is copied to SBUF
        # right after the last accumulation so the next head's out-matmuls
        # only wait for that one DVE copy.
        pst = ctx.enter_context(tc.tile_pool(name="psum_st", bufs=2,
                                             space="PSUM"))
        pout = ctx.enter_context(tc.tile_pool(name="psum_out", bufs=1,
                                              space="PSUM"))
        prp = ctx.enter_context(tc.tile_pool(name="psum_rep", bufs=1,
                                             space="PSUM"))
        pwo = ctx.enter_context(tc.tile_pool(name="psum_wo", bufs=1,
                                             space="PSUM"))

        pending = []

        def emit_pending():
            if pending:
                pending.pop(0)()

        proj_pending = [(1, 1, jd) for jd in range(HW // 512)]
        proj_pending += [(0, 1, 0), (0, 1, 1)]
        proj_pending += [(0, 0, jd) for jd in range(2, HW // 512)]
        proj_pending += [(0, 1, jd) for jd in range(2, HW // 512)]

        def make_divide(pocp, pb, hs, d0):
            def emit():
                rc = psmall.tile([1, DJ], F16, tag="rc", name="rc")
                # f16 reciprocal is safe here: denominators are O(100)
                # sums of exp(S-4), far from both f16 range limits
                with nc.allow_low_precision("softmax denom, O(100) range"):
                    nc.vector.reciprocal(rc[:], pocp[HD:HD + 1, :])
                for s in range(DJ // 512):
                    rp = prp.tile([HD, 512], F32, tag="rp", name="rp")
                    nc.tensor.matmul(
                        rp[:], ones_sb[:, :HD], rc[:, s * 512:(s + 1) * 512],
                        start=True, stop=True)
                    rps = psmall.tile([HD, 512], F32, tag="rps", name="rps")
                    nc.vector.tensor_copy(rps[:], rp[:])
                    nc.vector.tensor_mul(
                        ao_sb[pb:pb + HD, hs,
                              d0 + s * 512:d0 + (s + 1) * 512],
                        pocp[0:HD, s * 512:(s + 1) * 512], rps[:])
            return emit

        def make_wo(d0):
            def emit():
                # Wo projection + bias (rank-1 matmul row) + residual.
                # pso alternates between the wo bank and the (idle between
                # divides) rp bank so consecutive chunks overlap.
                for i in range(NSUB):
                    ot = psmall.tile([P, DJ], F16, tag="ot", bufs=2,
                                     name="ot")
                    for s in range(DJ // 512):
                        sl = slice(d0 + s * 512, d0 + (s + 1) * 512)
                        pso = pwo.tile([P, 512], F32, tag="wo", name="pso")
                        for t in range(NSUB):
                            nc.tensor.matmul(
                                pso[:], wo_sb[:, t, i * P:(i + 1) * P],
                                ao_sb[:, t, sl],
                                start=(t == 0), stop=False)
                        nc.tensor.matmul(
                            pso[:], bo_sb[:, i * P:(i + 1) * P],
                            ones512_sb[:], start=False, stop=True)
                        nc.vector.tensor_add(
                            ot[:, s * 512:(s + 1) * 512], pso[:],
                            x_sb[:, i, sl])
                    nc.sync.dma_start(
                        out_view[:, i, d0:d0 + DJ], ot[:])
            return emit

        for dj in range(NDJ):
            d0 = dj * DJ
            for h in range(NH):
                pb = (h % 2) * HD        # partition base for this head
                hs = h // 2              # channel subtile
                q_head = q_sb[pb:pb + HD, hs, d0:d0 + DJ]
                po = pout.tile([HD + 1, DJ], F32, tag="po", name="po")
                pts = []

                def out_mms(ec, po=po, pts=pts, h=h):
                    vl = vt_sb[:, ec].rearrange("p h c -> p (h c)")
                    for s in range(DJ // 512):
                        nc.tensor.matmul(
                            po[:, s * 512:(s + 1) * 512],
                            vl[:, 66 * h:66 * h + HD + 1],
                            pts[ec][:, s * 512:(s + 1) * 512],
                            start=(ec == 0), stop=(ec == NE - 1))

                for ec in range(NE):
                    st = pst.tile([P, DJ], F32, tag="st", name="st")
                    lhsT = k_sb[pb:pb + HD, hs, ec * P:(ec + 1) * P]
                    for s in range(DJ // 512):
                        nc.tensor.matmul(
                            st[:, s * 512:(s + 1) * 512], lhsT,
                            q_head[:, s * 512:(s + 1) * 512],
                            start=True, stop=True)
                    pt = ptp.tile([P, DJ], F16, tag="pt", name="pt")
                    nc.scalar.activation(pt[:], st[:], ACTF.Exp,
                                         bias=shift_sb[:])
                    pts.append(pt)
                    if ec > 0:
                        out_mms(ec - 1)
                    if ec in (5, 9):
                        emit_pending()
                    elif ec % 2 == 0 and ec > 0 and proj_pending:
                        T, i, jd = proj_pending.pop(0)
                        proj_chunk(T, i, jd, pwo, "wo")
                out_mms(NE - 1)

                # free po fast: one DVE copy to SBUF; divide runs deferred
                pocp = psmall.tile([HD + 1, DJ], F32, tag="pocp",
                                   name="pocp")
                nc.vector.tensor_copy(pocp[:], po[:])
                pending.append(make_divide(pocp, pb, hs, d0))
            pending.append(make_wo(d0))
        while pending:
            emit_pending()


_CACHE = {}


def _get_module():
    if "nc" not in _CACHE:
        _CACHE["nc"] = build_module()
    return _CACHE["nc"]


def make_in_maps(inputs):
    x = np.asarray(inputs["x"], np.float32).reshape(B, C, HW)
    cx = np.asarray(inputs["context"], np.float32).reshape(B, C, HW)
    scale = 1.0 / np.sqrt(HD)
    ws = []
    for key, sc in (("Wq", scale), ("Wk", 1.0), ("Wv", 1.0), ("Wo", 1.0)):
        w = np.asarray(inputs[key], np.float32)
        ws.append((w.T * sc).astype(np.float16).ravel())
    w_flat = np.concatenate(ws)
    gq_w = np.asarray(inputs["gn_q_w"], np.float32)
    gq_b = np.asarray(inputs["gn_q_b"], np.float32)
    gc_w = np.asarray(inputs["gn_ctx_w"], np.float32)
    gc_b = np.asarray(inputs["gn_ctx_b"], np.float32)
    gnp = np.stack([gq_w, gq_b, gc_w, gc_b], axis=-1).reshape(NSUB, P, 4)
    gnp_flat = gnp.transpose(1, 0, 2).astype(np.float16).ravel()
    bo_flat = (np.asarray(inputs["bo"], np.float32)
               .reshape(NSUB, P).T.astype(np.float16).ravel())
    gsel = np.zeros((P, 16), np.float16)
    for p in range(P):
        gsel[p, p // CH_PER_G] = 1
    tail = np.concatenate([gnp_flat, bo_flat, gsel.ravel(),
                           gsel.T.astype(np.float16).ravel()])

    in_maps = []
    for b in range(N_CORES):
        blob = np.empty(BLOB_N, np.float16)
        blob[OFF_X:OFF_X + C * HW] = x[b].astype(np.float16).ravel()
        blob[OFF_CTX:OFF_CTX + C * HW] = cx[b].astype(np.float16).ravel()
        blob[OFF_W:OFF_GNP] = w_flat
        blob[OFF_GNP:] = tail
        in_maps.append({"blob": blob})
    return in_maps


def assemble(results):
    outf = np.empty((B, C, HW), np.float32)
    for b in range(N_CORES):
        outf[b] = results[b]["out"].astype(np.float32)
    return outf.reshape(B, C, 64, 64)


def kernel(**inputs) -> np.ndarray:
    nc = _get_module()
    in_maps = make_in_maps(inputs)
    res = run_bass_kernel_spmd(nc, in_maps, core_ids=list(range(N_CORES)))
    return assemble(res.results)


# revision 44
# speedup vs baseline: 1.0772x; 1.0772x over previous
"""Trainium2 Bass kernel for a CrossAttentionBlock.

Reference computation (B=4, C=256, H=W=64, 4 heads, head_dim=64):
  q = Wq @ GN(x);  k = Wk @ GN(ctx);  v = Wv @ ctx        (1x1 convs)
  attn = softmax(q^T k / sqrt(hd))  per (batch, head)
  out  = x + Wo @ (v @ attn^T) + bo

Interface: 4 cores, one full batch element per core. All per-core inputs
are packed into a SINGLE f16 blob tensor (the PJRT-over-axon execute path
pays a large per-argument per-call cost, and per-byte transfer cost, so
one small blob argument beats twelve f32 tensors by ~10x end to end).
Output is a single f16 [C, HW] tensor per core.

On-chip algorithm per core (matmuls in f16, fp32 PSUM accum):
  - GroupNorm stats via ones-selector matmul (per-group sums of x and x^2),
    expanded back to per-channel affine (a, d) with a selector matmul.
  - The GN affine is FOLDED into the Q/K projections: Wq' = Wq. diag(a)
    (per-partition row scale of the resident weights) and a rank-1 bias
    qb = Wq @ d computed with an N=1 matmul, so the normalized tensors are
    never materialized.
  - S^T[e, d] = sum_c k[c,e] q[c,d] per 128-wide e-chunk so the attn @ v
    matmul needs no transpose: lhsT = [v^T | ones], so PSUM row 64
    accumulates the softmax denominator for free.  exp(S - 4) on the
    scalar engine keeps the f16 range safe (|S| <~ 8 for these inputs;
    the -4 shift cancels in the softmax ratio).
  - softmax divide via DVE reciprocal + a rank-1 fp32 ones matmul that
    replicates the per-column reciprocal across partitions.
"""

import sys

if "/opt/trn_rl_repo" not in sys.path:
    sys.path.insert(0, "/opt/trn_rl_repo")

import copy
from contextlib import ExitStack

import numpy as np

import bass_rust
import concourse.bass as bass
import concourse.mybir as mybir
import concourse.tile as tile
from concourse.bass_utils import run_bass_kernel_spmd
from concourse.vector_clock import ScopedClock

F32 = mybir.dt.float32
F16 = mybir.dt.float16

N_CORES = 4
B, C, HW = 4, 256, 4096
NH, HD = 4, 64          # heads, head dim
P = 128                 # partitions
NSUB = C // P           # channel subtiles (2)
GROUPS = 32             # groupnorm groups (16 per channel-subtile)
CH_PER_G = C // GROUPS  # 8
GN_N = CH_PER_G * HW    # elements per group (32768)
EPS = 1e-5
DJ = 1024               # main-loop d-chunk (exp granularity)
NDJ = HW // DJ          # 4
NE = HW // P            # 32 e-chunks
EXP_SHIFT = -4.0        # exp(S + EXP_SHIFT); cancels in softmax ratio
ALU = mybir.AluOpType
ACTF = mybir.ActivationFunctionType
AXX = mybir.AxisListType.X

# ---- blob layout (f16 element offsets) ----
OFF_X = 0                          # x    [C, HW] as (t p d)
OFF_CTX = OFF_X + C * HW           # ctx  [C, HW] as (t p d)
OFF_W = OFF_CTX + C * HW           # wqt, wkt, wvt, wot [C, C] as (t p o)
OFF_GNP = OFF_W + 4 * C * C        # [P, NSUB, 4] (p t f)
OFF_BO = OFF_GNP + P * NSUB * 4    # [P, NSUB] (p t)
OFF_GSEL = OFF_BO + P * NSUB       # [P, 16] (p g)
OFF_SELT = OFF_GSEL + P * 16       # [16, P] (g p)
BLOB_N = OFF_SELT + 16 * P


class SplitDrainTileContext(tile.TileContext):
    """TileContext whose exit drain splits sem waits across multiple Drain
    instructions — the walrus build in this container rejects >2 sync waits
    on a single Drain ("Too many sync wait commands")."""

    def _drain_and_barrier(self, tick_clock, wait_clock):
        drain_inst = self.nc.sync.drain()
        wait_clock.add_sem_waits(
            drain_inst.ins, ScopedClock({None: tick_clock.global_clock})
        )
        si = drain_inst.ins.sync_info
        if si is not None and si.on_wait and len(si.on_wait) > 1:
            waits = list(si.on_wait)
            si.on_wait = waits[:1]
            drain_inst.ins.sync_info = si
            for w in waits[1:]:
                extra = self.nc.sync.drain()
                extra.ins.sync_info = bass_rust.SyncInfo(on_wait=[w], on_update=[])
        self.nc.all_engine_barrier()
        popped = self.nc._tile_sem_poison_stack.pop()
        assert popped is self._sem_poison
        self.nc.clear_and_free_semaphores(list(self.sems.allocated().values()))
        self.nc.all_engine_barrier()


_NOP_TMPL = []


def _nop_template():
    if not _NOP_TMPL:
        tb = bass.Bass()
        with tb.bb("t"):
            _NOP_TMPL.append(copy.copy(tb.vector.nop().ins))
    return _NOP_TMPL[0]


def _split_excess_waits(nc, limit=1):
    """This container's walrus rejects instructions carrying more than ~2
    sync-wait commands. Spill excess waits onto same-engine NoOps inserted
    just before the overloaded instruction (waiting earlier on the same
    engine is semantics-preserving; NoOps have no dependents, so no cycles
    can form)."""
    tmpl = _nop_template()
    n = 0

    def fix(blk):
        nonlocal n
        if hasattr(blk, "instructions"):
            out = []
            changed = False
            for inst in blk.instructions:
                si = inst.sync_info
                ow = list(si.on_wait) if (si is not None and si.on_wait) else []
                lim = 1 if ("DMA" in inst.opcode or inst.opcode == "Drain") \
                    else limit
                if len(ow) > lim:
                    changed = True
                    for w in ow[:-lim]:
                        sp = copy.copy(tmpl)
                        n += 1
                        sp.name = f"I-wsp-{n}"
                        sp.engine = inst.engine
                        sp.sync_info = bass_rust.SyncInfo(on_wait=[w],
                                                          on_update=[])
                        out.append(sp)
                    si.on_wait = ow[-lim:]
                    inst.sync_info = si
                out.append(inst)
            if changed:
                blk.instructions = out
        for sub in getattr(blk, "blocks", []) or []:
            fix(sub)

    for f in nc.m.functions:
        for blk in f.blocks:
            fix(blk)
    return n


def build_module(iters: int = 1) -> bass.Bass:
    """iters > 1 repeats the whole body (for slope-based device timing)."""
    nc = bass.Bass()
    blob = nc.dram_tensor("blob", [BLOB_N], F16, kind="ExternalInput")
    out = nc.dram_tensor("out", [C, HW], F16, kind="ExternalOutput")
    with SplitDrainTileContext(nc) as tc:
        for _ in range(iters):
            _emit(nc, tc, blob, out)
    _split_excess_waits(nc)
    return nc


def _emit(nc, tc, blob, out_dr):
    x_view = blob[OFF_X:OFF_X + C * HW].rearrange("(t p d) -> p t d",
                                                  t=NSUB, p=P)
    ctx_view = blob[OFF_CTX:OFF_CTX + C * HW].rearrange("(t p d) -> p t d",
                                                        t=NSUB, p=P)
    w_views = [
        blob[OFF_W + i * C * C:OFF_W + (i + 1) * C * C].rearrange(
            "(t p o) -> p t o", t=NSUB, p=P)
        for i in range(4)
    ]
    gnp_view = blob[OFF_GNP:OFF_GNP + P * NSUB * 4].rearrange(
        "(p t f) -> p t f", p=P, t=NSUB)
    bo_view = blob[OFF_BO:OFF_BO + P * NSUB].rearrange("(a o) -> a o", a=1)
    gsel_view = blob[OFF_GSEL:OFF_GSEL + P * 16].rearrange("(p g) -> p g", p=P)
    selt_view = blob[OFF_SELT:OFF_SELT + 16 * P].rearrange("(g p) -> g p", g=16)
    out_view = out_dr[:].rearrange("(t p) d -> p t d", p=P)

    with ExitStack() as ctx:
        pw = ctx.enter_context(tc.tile_pool(name="pw", bufs=1))
        pmain = ctx.enter_context(tc.tile_pool(name="pmain", bufs=1))
        ptp = ctx.enter_context(tc.tile_pool(name="ptp", bufs=4))
        psmall = ctx.enter_context(tc.tile_pool(name="psmall", bufs=2))

        # ---- small constants (gsel first: it gates the stats matmuls) ----
        gsel_sb = pw.tile([P, 16], F16, name="gsel_sb")
        nc.sync.dma_start(gsel_sb[:], gsel_view)
        gnp_sb = pw.tile([P, NSUB, 4], F16, name="gnp_sb")
        nc.gpsimd.dma_start(gnp_sb[:], gnp_view)
        selt_sb = pw.tile([16, P], F16, name="selt_sb")
        nc.gpsimd.dma_start(selt_sb[:], selt_view)
        bo_sb = pw.tile([1, C], F16, name="bo_sb")
        nc.scalar.dma_start(bo_sb[:], bo_view)
        wq_sb = pw.tile([P, NSUB, C], F16, name="wq_sb")
        wk_sb = pw.tile([P, NSUB, C], F16, name="wk_sb")
        wv_sb = pw.tile([P, NSUB, C], F16, name="wv_sb")
        wo_sb = pw.tile([P, NSUB, C], F16, name="wo_sb")
        wmup_sb = pw.tile([P, P], F16, name="wmup_sb")
        nc.vector.memset(wmup_sb[:], 0.0)
        ones_sb = pw.tile([1, HD], F16, name="ones_sb")
        nc.vector.memset(ones_sb[:], 1.0)
        ones512_sb = pw.tile([1, 512], F16, name="ones512_sb")
        nc.vector.memset(ones512_sb[:], 1.0)
        eps_sb = pw.tile([16, 1], F32, name="eps_sb")
        nc.vector.memset(eps_sb[:], EPS)
        shift_sb = pw.tile([P, 1], F32, name="shift_sb")
        nc.vector.memset(shift_sb[:], EXP_SHIFT)

        # ---- persistent activations, chunked across three DMA queues so
        # GN stats can start early on partial data ----
        x_sb = pmain.tile([P, NSUB, HW], F16, name="x_sb")
        cb_sb = pmain.tile([P, NSUB, HW], F16, name="cb_sb")
        big = [(x_sb, x_view, 0), (x_sb, x_view, 1),
               (cb_sb, ctx_view, 0), (cb_sb, ctx_view, 1),
               (x_sb, x_view, 2), (x_sb, x_view, 3),
               (cb_sb, ctx_view, 2), (cb_sb, ctx_view, 3)]
        queues = [nc.sync, nc.gpsimd, nc.scalar]
        for qi, (dst, view, cch) in enumerate(big):
            sl = slice(cch * 1024, (cch + 1) * 1024)
            queues[qi % 3].dma_start(dst[:, :, sl], view[:, :, sl])
        # weights after the activations (first needed ~10us in)
        for i, w_sb in enumerate((wq_sb, wk_sb, wv_sb, wo_sb)):
            queues[i % 3].dma_start(w_sb[:], w_views[i])
        q_sb = pmain.tile([P, NSUB, HW], F16, name="q_sb")
        k_sb = pmain.tile([P, NSUB, HW], F16, name="k_sb")
        vt_sb = pmain.tile([P, NE, NH, 66], F16, name="vt_sb")
        ao_sb = pmain.tile([P, NSUB, HW], F16, name="ao_sb")
        stats_sb = pmain.tile([16, 8], F32, name="stats_sb")
        grp_sb = pmain.tile([P, 8], F32, name="grp_sb")
        aff_sb = pmain.tile([P, 2, NSUB, 2], F32, name="aff_sb")
        dvec_sb = pmain.tile([P, 2, NSUB, 1], F16, name="dvec_sb")
        qb_sb = pmain.tile([P, 2, NSUB], F32, name="qb_sb")

        # ============ prep phase: GN stats, fold affine, Q/K/V^T ============
        with ExitStack() as prep:
            pps = prep.enter_context(
                tc.tile_pool(name="pps", bufs=2, space="PSUM"))
            pchunk = prep.enter_context(tc.tile_pool(name="pchunk", bufs=2))

            # keep the PE continuously busy from t=0 so it reaches the full
            # 2.4GHz p-state before the real prep matmuls arrive
            for w in range(20):
                junk = pps.tile([P, 512], F32, tag="qk", name="junk")
                nc.tensor.matmul(junk[:, :P], wmup_sb[:], wmup_sb[:, :P],
                                 start=True, stop=True)

            def gn_stats(src_sb, tensor_idx):
                """Per-group sums of x and x^2 via the selector matmul,
                streamed in [P, NSUB, 512] chunks out of resident f16 data.
                Column-reduces split across DVE and Act to shorten the
                serial chain."""
                nj = HW // 512
                ps = {
                    (t, k): pps.tile([16, 512], F32, tag="stat", bufs=4,
                                     name=f"ps_stat{tensor_idx}{t}{k}")
                    for t in range(NSUB) for k in range(2)
                }
                for j in range(nj):
                    sl = slice(j * 512, (j + 1) * 512)
                    sq = pchunk.tile([P, NSUB, 512], F16, tag="sq", name="sq")
                    for t in range(NSUB):
                        nc.vector.tensor_mul(sq[:, t], src_sb[:, t, sl],
                                             src_sb[:, t, sl])
                        nc.tensor.matmul(ps[(t, 0)][:], gsel_sb[:],
                                         src_sb[:, t, sl],
                                         start=(j == 0), stop=(j == nj - 1))
                        nc.tensor.matmul(ps[(t, 1)][:], gsel_sb[:], sq[:, t],
                                         start=(j == 0), stop=(j == nj - 1))
                for t in range(NSUB):
                    for k in range(2):
                        # block layout: sums in cols [0:4], sq-sums [4:8]
                        col = 2 * tensor_idx + t + 4 * k
                        dst = stats_sb[:, col:col + 1]
                        if k == 0:
                            nc.vector.reduce_sum(dst, ps[(t, k)][:], axis=AXX)
                        else:
                            nc.scalar.activation(ps[(t, k)][:], ps[(t, k)][:],
                                                 ACTF.Copy, accum_out=dst)

            # ctx stats first: the K-side chain is the critical path
            gn_stats(cb_sb, 1)
            gn_stats(x_sb, 0)

            packed = psmall.tile([16, 8], F32, name="packed")
            packed16 = psmall.tile([16, 8], F16, tag="p16", name="packed16")
            inv_n = 1.0 / GN_N

            # wide mean/rstd: all four (tensor, subtile) stat groups in
            # single [16,4] ops — one sqrt + one reciprocal total, instead
            # of a long per-group DVE<->Act ping-pong chain
            nc.vector.tensor_scalar_mul(packed[:, 0:4], stats_sb[:, 0:4],
                                        inv_n)
            nc.vector.tensor_scalar_mul(packed[:, 4:8], stats_sb[:, 4:8],
                                        inv_n)
            m2 = psmall.tile([16, 4], F32, tag="m2", name="m2")
            nc.vector.tensor_mul(m2[:], packed[:, 0:4], packed[:, 0:4])
            nc.vector.tensor_sub(packed[:, 4:8], packed[:, 4:8], m2[:])
            nc.scalar.activation(packed[:, 4:8], packed[:, 4:8], ACTF.Sqrt,
                                 bias=eps_sb[:])
            nc.vector.reciprocal(packed[:, 4:8], packed[:, 4:8])
            # expand groups -> channels: grp block layout [means | rstds]
            nc.vector.tensor_copy(packed16[:], packed[:])
            psg = pps.tile([P, 8], F32, tag="exp", bufs=1, name="psg")
            nc.tensor.matmul(psg[:], selt_sb[:], packed16[:], start=True,
                             stop=True)
            nc.vector.tensor_copy(grp_sb[:], psg[:])

            def gn_chain(T, w_sb):
                """affine -> rank-1 bias -> fold for one tensor side.  The
                rank-1 bias matmul reads the UNSCALED weights, so it must
                precede the in-place fold; everything else is ordered to
                unblock the projection matmuls as early as possible."""
                for t in range(NSUB):
                    idx = 2 * T + t
                    wcol = 0 if T == 0 else 2
                    a = aff_sb[:, T, t, 0:1]
                    d = aff_sb[:, T, t, 1:2]
                    nc.vector.tensor_mul(
                        a, gnp_sb[:, t, wcol:wcol + 1],
                        grp_sb[:, 4 + idx:5 + idx])
                    tmp = psmall.tile([P, 1], F32, tag="afft", name="afft")
                    nc.vector.tensor_mul(tmp[:], grp_sb[:, idx:idx + 1], a)
                    nc.vector.tensor_sub(
                        d, gnp_sb[:, t, wcol + 1:wcol + 2], tmp[:])
                    nc.vector.tensor_copy(dvec_sb[:, T, t], d)
                # rank-1 GN bias through the UNSCALED weights: qb = W^T d
                pb = pps.tile([P, NSUB], F32, tag="bias", bufs=1,
                              name=f"pbias{T}")
                for i in range(NSUB):
                    for t in range(NSUB):
                        nc.tensor.matmul(
                            pb[:, i:i + 1], w_sb[:, t, i * P:(i + 1) * P],
                            dvec_sb[:, T, t],
                            start=(t == 0), stop=(t == NSUB - 1))
                # fold the GN scale into the weights (per-partition rows)
                # before the qb PSUM copy: the projections wait on the fold
                for t in range(NSUB):
                    nc.vector.tensor_scalar(
                        w_sb[:, t], w_sb[:, t], aff_sb[:, T, t, 0:1], None,
                        op0=ALU.mult)
                nc.vector.tensor_copy(qb_sb[:, T], pb[:])

            gn_chain(1, wk_sb)

            # ---- K/Q projection chunks the main loop needs up front:
            # K(i=0) fully and Q(i=0) first two chunks (heads 0/1, dj=0) ----
            def proj_chunk(T, i, jd, pool, tag, bufs=1, on_act=False):
                w_sb = wq_sb if T == 0 else wk_sb
                src = x_sb if T == 0 else cb_sb
                dst = q_sb if T == 0 else k_sb
                psq = pool.tile([P, 512], F32, tag=tag, bufs=bufs, name="psq")
                for t in range(NSUB):
                    nc.tensor.matmul(
                        psq[:], w_sb[:, t, i * P:(i + 1) * P],
                        src[:, t, jd * 512:(jd + 1) * 512],
                        start=(t == 0), stop=(t == NSUB - 1))
                out_sl = dst[:, i, jd * 512:(jd + 1) * 512]
                if on_act:
                    # prep only: the idle Act engine does the bias-add so
                    # the DVE never serializes the chain to the first exp
                    nc.scalar.activation(out_sl, psq[:], ACTF.Identity,
                                         bias=qb_sb[:, T, i:i + 1])
                else:
                    nc.vector.tensor_scalar(
                        out_sl, psq[:], qb_sb[:, T, i:i + 1], None,
                        op0=ALU.add)

            # K(i=0) right after the ctx chain (critical path to the first
            # exp), then the x-side chain and Q's first two chunks.  psq
            # tiles use the stat banks (free once the reduces are done).
            gn_chain(0, wq_sb)
            for jd in range(HW // 512):
                proj_chunk(1, 0, jd, pps, "stat", bufs=4)
            for jd in range(2):
                proj_chunk(0, 0, jd, pps, "stat", bufs=4, on_act=True)

            # prewarm the Exp activation table; ones columns of vt
            dmy = psmall.tile([1, 1], F16, tag="dmy", name="dmy")
            nc.scalar.activation(dmy[:], shift_sb[0:1, :], ACTF.Exp)
            nc.vector.memset(vt_sb[:, :, :, 64:65], 1.0)
            nc.vector.memset(vt_sb[:, :, :, 65:66], 0.0)

        # ================= attention main loop =================
        # Remaining K/Q projection chunks stream into the early e-chunk
        # iterations (even ecs) where the PE has slack vs the Act engine.
        # Ordered so each chunk lands well before its consumer head.
        # Divide/Wo work is DEFERRED into the next head's e-chunk stream so
        # the Act engine (the bottleneck) never waits on the PE-side pieces
        # of the softmax divide at head boundaries.  po is copied to SBUF
        # right after the last accumulation so the next head's out-matmuls
        # only wait for that one DVE copy.
        pst = ctx.enter_context(tc.tile_pool(name="psum_st", bufs=2,
                                             space="PSUM"))
        pout = ctx.enter_context(tc.tile_pool(name="psum_out", bufs=1,
                                              space="PSUM"))
        prp = ctx.enter_context(tc.tile_pool(name="psum_rep", bufs=1,
                                             space="PSUM"))
        pwo = ctx.enter_context(tc.tile_pool(name="psum_wo", bufs=1,
                                             space="PSUM"))

        pending = []

        def emit_pending():
            if pending:
                pending.pop(0)()

        proj_pending = [(1, 1, jd) for jd in range(HW // 512)]
        proj_pending += [(0, 1, 0), (0, 1, 1)]
        proj_pending += [(0, 0, jd) for jd in range(2, HW // 512)]
        proj_pending += [(0, 1, jd) for jd in range(2, HW // 512)]

        # V^T projection streams through the first head's e-chunks: chunk ec
        # lands exactly one iteration before its consumer out-matmul, the
        # PSUM->SBUF copy rides the idle DVE, and the Act engine starts
        # exp'ing ~15us earlier than a prep-phase V would allow
        def v_chunk(ec):
            psv = pwo.tile([P, 512], F32, tag="wo", name="psv")
            for t in range(NSUB):
                nc.tensor.matmul(
                    psv[:, :C], cb_sb[:, t, ec * P:(ec + 1) * P],
                    wv_sb[:, t, :],
                    start=(t == 0), stop=(t == NSUB - 1))
            nc.vector.tensor_copy(
                vt_sb[:, ec, :, 0:64],
                psv[:, :C].rearrange("p (h c) -> p h c", c=64))

        def make_divide(pocp, pb, hs, d0):
            def emit():
                rc = psmall.tile([1, DJ], F16, tag="rc", name="rc")
                # f16 reciprocal is safe here: denominators are O(100)
                # sums of exp(S-4), far from both f16 range limits
                with nc.allow_low_precision("softmax denom, O(100) range"):
                    nc.vector.reciprocal(rc[:], pocp[HD:HD + 1, :])
                for s in range(DJ // 512):
                    rp = prp.tile([HD, 512], F32, tag="rp", name="rp")
                    nc.tensor.matmul(
                        rp[:], ones_sb[:, :HD], rc[:, s * 512:(s + 1) * 512],
                        start=True, stop=True)
                    rps = psmall.tile([HD, 512], F32, tag="rps", name="rps")
                    nc.vector.tensor_copy(rps[:], rp[:])
                    nc.vector.tensor_mul(
                        ao_sb[pb:pb + HD, hs,
                              d0 + s * 512:d0 + (s + 1) * 512],
                        pocp[0:HD, s * 512:(s + 1) * 512], rps[:])
            return emit

        def make_wo_chunks(d0, use_rp=False):
            # One closure per (i, s) output chunk so the Wo work spreads
            # across several e-chunk slots instead of one ~5us PE burst.
            # use_rp also claims the rp bank (safe at the final-dj tail).
            ots = {}

            def chunk(i, s, n):
                def emit():
                    if i not in ots:
                        ots[i] = psmall.tile([P, DJ], F16, tag="ot", bufs=2,
                                             name="ot")
                    ot = ots[i]
                    sl = slice(d0 + s * 512, d0 + (s + 1) * 512)
                    if use_rp and n % 2 == 1:
                        pso = prp.tile([P, 512], F32, tag="rp", name="pso2")
                    else:
                        pso = pwo.tile([P, 512], F32, tag="wo", name="pso")
                    for t in range(NSUB):
                        nc.tensor.matmul(
                            pso[:], wo_sb[:, t, i * P:(i + 1) * P],
                            ao_sb[:, t, sl],
                            start=(t == 0), stop=False)
                    nc.tensor.matmul(
                        pso[:], bo_sb[:, i * P:(i + 1) * P],
                        ones512_sb[:], start=False, stop=True)
                    nc.vector.tensor_add(
                        ot[:, s * 512:(s + 1) * 512], pso[:], x_sb[:, i, sl])
                    if s == DJ // 512 - 1:
                        nc.sync.dma_start(out_view[:, i, d0:d0 + DJ], ot[:])
                return emit

            return [chunk(i, s, n)
                    for n, (i, s) in enumerate(
                        (i, s) for i in range(NSUB)
                        for s in range(DJ // 512))]

        for dj in range(NDJ):
            d0 = dj * DJ
            for h in range(NH):
                pb = (h % 2) * HD        # partition base for this head
                hs = h // 2              # channel subtile
                q_head = q_sb[pb:pb + HD, hs, d0:d0 + DJ]
                po = pout.tile([HD + 1, DJ], F32, tag="po", name="po")
                pts = []

                def out_mms(ec, po=po, pts=pts, h=h):
                    vl = vt_sb[:, ec].rearrange("p h c -> p (h c)")
                    for s in range(DJ // 512):
                        nc.tensor.matmul(
                            po[:, s * 512:(s + 1) * 512],
                            vl[:, 66 * h:66 * h + HD + 1],
                            pts[ec][:, s * 512:(s + 1) * 512],
                            start=(ec == 0), stop=(ec == NE - 1))

                for ec in range(NE):
                    st = pst.tile([P, DJ], F32, tag="st", name="st")
                    lhsT = k_sb[pb:pb + HD, hs, ec * P:(ec + 1) * P]
                    for s in range(DJ // 512):
                        nc.tensor.matmul(
                            st[:, s * 512:(s + 1) * 512], lhsT,
                            q_head[:, s * 512:(s + 1) * 512],
                            start=True, stop=True)
                    pt = ptp.tile([P, DJ], F16, tag="pt", name="pt")
                    nc.scalar.activation(pt[:], st[:], ACTF.Exp,
                                         bias=shift_sb[:])
                    pts.append(pt)
                    if dj == 0 and h == 0:
                        v_chunk(ec)
                    if ec > 0:
                        out_mms(ec - 1)
                    if ec in (5, 9, 13, 17, 21):
                        emit_pending()
                    elif (ec % 2 == 0 and ec > 0 and proj_pending
                          and not (dj == 0 and h == 0)):
                        T, i, jd = proj_pending.pop(0)
                        proj_chunk(T, i, jd, pwo, "wo")
                out_mms(NE - 1)

                # free po fast: one DVE copy to SBUF; divide runs deferred
                pocp = psmall.tile([HD + 1, DJ], F32, tag="pocp",
                                   name="pocp")
                nc.vector.tensor_copy(pocp[:], po[:])
                pending.append(make_divide(pocp, pb, hs, d0))
            pending.extend(make_wo_chunks(d0, use_rp=(dj == NDJ - 1)))
        while pending:
            emit_pending()


_CACHE = {}


def _get_module():
    if "nc" not in _CACHE:
        _CACHE["nc"] = build_module()
    return _CACHE["nc"]


def make_in_maps(inputs):
    x = np.asarray(inputs["x"], np.float32).reshape(B, C, HW)
    cx = np.asarray(inputs["context"], np.float32).reshape(B, C, HW)
    scale = 1.0 / np.sqrt(HD)
    ws = []
    for key, sc in (("Wq", scale), ("Wk", 1.0), ("Wv", 1.0), ("Wo", 1.0)):
        w = np.asarray(inputs[key], np.float32)
        ws.append((w.T * sc).astype(np.float16).ravel())
    w_flat = np.concatenate(ws)
    gq_w = np.asarray(inputs["gn_q_w"], np.float32)
    gq_b = np.asarray(inputs["gn_q_b"], np.float32)
    gc_w = np.asarray(inputs["gn_ctx_w"], np.float32)
    gc_b = np.asarray(inputs["gn_ctx_b"], np.float32)
    gnp = np.stack([gq_w, gq_b, gc_w, gc_b], axis=-1).reshape(NSUB, P, 4)
    gnp_flat = gnp.transpose(1, 0, 2).astype(np.float16).ravel()
    bo_flat = (np.asarray(inputs["bo"], np.float32)
               .reshape(NSUB, P).T.astype(np.float16).ravel())
    gsel = np.zeros((P, 16), np.float16)
    for p in range(P):
        gsel[p, p // CH_PER_G] = 1
    tail = np.concatenate([gnp_flat, bo_flat, gsel.ravel(),
                           gsel.T.astype(np.float16).ravel()])

    in_maps = []
    for b in range(N_CORES):
        blob = np.empty(BLOB_N, np.float16)
        blob[OFF_X:OFF_X + C * HW] = x[b].astype(np.float16).ravel()
        blob[OFF_CTX:OFF_CTX + C * HW] = cx[b].astype(np.float16).ravel()
        blob[OFF_W:OFF_GNP] = w_flat
        blob[OFF_GNP:] = tail
        in_maps.append({"blob": blob})
    return in_maps


def assemble(results):
    outf = np.empty((B, C, HW), np.float32)
    for b in range(N_CORES):
        outf[b] = results[b]["out"].astype(np.float32)
    return outf.reshape(B, C, 64, 64)


def kernel(**inputs) -> np.ndarray:
    nc = _get_module()
    in_maps = make_in_maps(inputs)
    res = run_bass_kernel_spmd(nc, in_maps, core_ids=list(range(N_CORES)))
    return assemble(res.results)
